# revision 8
# baseline (speedup 1.0000x reference)
"""Distributed GQA attention kernel for 8 TRN2 NeuronCores (Bass/Tile).

Problem (hardcoded): B=2, S=2048, DM=1024, H=16 q-heads, KH=4 kv-heads, HD=64.
reference: out = softmax_causal((RoPE(x@wq) @ RoPE(x@wk)^T)/sqrt(HD)) @ (x@wv) @ wo

Sharding: core c in 0..7 -> batch b = c//4, kv-group g = c%4.
Each core computes q-heads [4g..4g+4), kv head g for batch b.  The 4 cores
of one batch then AllToAll their attention outputs (each core sends peer p
its rows for token-quarter p), and each core computes the o-projection of
its own 512-token quarter against the FULL wo, writing a [512, 1024] out
slice.  Host stacks the quarters into the full [2,2048,1024] output.

All matmuls run in bf16 with f32 PSUM accumulation.  Scores are computed
transposed ([k,q]) so the softmax denominator falls out of a ones-column in
the PV matmul; softmax skips max-subtraction (scores are O(3) for this
problem scale, well within fp32 exp range).  RoPE's rotate_half is done
with partition-offset vector muls (no permutation matmul); causality is
handled by issuing score matmuls only for q >= k plus one triangular mask
multiply on diagonal 128x128 blocks.  The softmax 1/Z broadcast runs on
the idle GpSimd/Pool engine (partition_broadcast) instead of a DRAM
round-trip.

_build(nrep=N) repeats the whole per-iteration body N times inside one NEFF
(used by the benchmark to amortize the ~80 ms axon dispatch overhead and
measure per-iteration HW time from the slope).
"""

import numpy as np
import ml_dtypes

import concourse.bass as bass
import concourse.bacc as bacc
import concourse.mybir as mybir
import concourse.tile as tile
from concourse import bass_utils

B, S, DM = 2, 2048, 1024
H, KH, HD = 16, 4, 64
NCORES = 8
TPG = 4            # tensor-parallel group size (cores per batch)
QH_PER_CORE = 4    # q heads per core
QR = QH_PER_CORE * HD   # 256 q rows per core
OSL = DM // TPG    # 256 out columns per core

F32 = mybir.dt.float32
BF16 = mybir.dt.bfloat16
FP = mybir.ActivationFunctionType

_CACHE = {}
PROFILE = False
LAST_RESULTS = None

ARRANGE = "a2a"


def _build(nrep=1, no_collective=False):
    SCB = 3   # "sc" psum slots
    nc = bacc.Bacc("TRN2", debug=False, enable_asserts=False,
                   num_devices=NCORES)

    xT = nc.dram_tensor("xT", [DM, S], BF16, kind="ExternalInput")
    wq = nc.dram_tensor("wq", [DM, QR], BF16, kind="ExternalInput")
    wk = nc.dram_tensor("wk", [DM, HD], BF16, kind="ExternalInput")
    wv = nc.dram_tensor("wv", [DM, HD], BF16, kind="ExternalInput")
    wo = nc.dram_tensor("wo", [DM, OSL], BF16, kind="ExternalInput")
    cosT = nc.dram_tensor("cosT", [128, S], BF16, kind="ExternalInput")
    sinT = nc.dram_tensor("sinT", [128, S], BF16, kind="ExternalInput")
    permT = nc.dram_tensor("permT", [128, 128], BF16, kind="ExternalInput")
    tri = nc.dram_tensor("tri", [128, 128], BF16, kind="ExternalInput")
    identd = nc.dram_tensor("ident", [64, 64], BF16, kind="ExternalInput")
    out = nc.dram_tensor("out", [S, OSL], F32, kind="ExternalOutput")

    groups = [[0, 1, 2, 3], [4, 5, 6, 7]]

    with tile.TileContext(nc) as tc:
        with tc.tile_pool(name="const", bufs=1) as constp, \
             tc.tile_pool(name="pers", bufs=1) as pers, \
             tc.tile_pool(name="work", bufs=1) as work, \
             tc.tile_pool(name="ps", bufs=1, space="PSUM") as psp, \
             tc.tile_pool(name="dram", bufs=1, space="DRAM") as dramp:

            # ---- constants (already bf16 in DRAM)
            def load_const(dram_t, rows, cols, cname, dt_out=BF16):
                t = constp.tile([rows, cols], dt_out, name=cname, tag=cname)
                nc.sync.dma_start(t[:], dram_t.ap())
                return t

            cos_sb = load_const(cosT, 128, S, "cos_sb")
            sin_sb = load_const(sinT, 128, S, "sin_sb")
            perm_sb = load_const(permT, 128, 128, "perm_sb")
            tri_sb = load_const(tri, 128, 128, "tri_sb")
            ident128 = constp.tile([128, 64], BF16, name="ident128",
                                   tag="ident128")
            nc.sync.dma_start(ident128[0:64, :], identd.ap())
            nc.sync.dma_start(ident128[64:128, :], identd.ap())

            # weights (once): per 128-row dm chunk.  wq|wk|wv per chunk,
            # and the FULL wo (this core's token quarter needs all of it).
            WCOLS = QR + HD + HD + OSL
            wq_sb, wkv_sb, wo_sb = [], [], []
            for c in range(8):
                wt = pers.tile([128, WCOLS], BF16, name=f"w_{c}",
                               tag=f"w_{c}")
                nc.sync.dma_start(wt[:, 0:QR],
                                  wq.ap()[128 * c:128 * c + 128, :])
                nc.sync.dma_start(wt[:, QR:QR + HD],
                                  wk.ap()[128 * c:128 * c + 128, :])
                nc.sync.dma_start(wt[:, QR + HD:QR + 2 * HD],
                                  wv.ap()[128 * c:128 * c + 128, :])
                nc.sync.dma_start(wt[:, QR + 2 * HD:],
                                  wo.ap()[128 * c:128 * c + 128, :])
                wq_sb.append(wt[:, 0:QR])
                wkv_sb.append(wt[:, QR:QR + 2 * HD])
                wo_sb.append(wt[:, QR + 2 * HD:])

            cc_in = [dramp.tile([QR, 1024], BF16, name=f"cc_in{qh}",
                                tag=f"cc_in{qh}")
                     for qh in range(2)]
            cc_out = [dramp.tile([TPG * QR, 1024], BF16,
                                 name=f"cc_out{qh}", tag=f"cc_out{qh}")
                      for qh in range(2)]

            def emit_tail1():
                attn_all = []
                for c2 in range(8):
                    t = work.tile([128, 1024], BF16, tag=f"aall_{c2}",
                                  bufs=2)
                    nc.sync.dma_start(
                        t[:], cc_out[1][128 * c2:128 * c2 + 128, :])
                    attn_all.append(t)
                for tc8 in range(8):
                    ps = psp.tile([128, OSL], F32, tag="sc", bufs=SCB)
                    for c2 in range(8):
                        nc.tensor.matmul(
                            ps[:],
                            attn_all[c2][:, 128 * tc8:128 * tc8 + 128],
                            wo_sb[c2],
                            start=(c2 == 0), stop=(c2 == 7))
                    ot = work.tile([128, OSL], F32, tag="ot", bufs=2)
                    nc.vector.tensor_copy(ot[:], ps[:])
                    nc.sync.dma_start(
                        out.ap()[1024 + 128 * tc8:1024 + 128 * tc8 + 128,
                                 :],
                        ot[:])

            tail_pending = [False]

            for _rep in range(nrep):
                # ---- x -> bf16 (transposed layout [dm, tokens])
                xbf = []
                for c in range(8):
                    t = work.tile([128, S], BF16, tag=f"xbf_{c}")
                    for th in range(2):
                        nc.sync.dma_start(
                            t[:, 1024 * th:1024 * th + 1024],
                            xT.ap()[128 * c:128 * c + 128,
                                    1024 * th:1024 * th + 1024])
                    xbf.append(t)

                q_rot = [work.tile([128, S], BF16, name=f"qrot_{rc}",
                                   tag=f"qrot_{rc}")
                         for rc in range(2)]
                k_rot = work.tile([128, S], BF16, tag="krot")
                vT_sb = work.tile([128, S], BF16, tag="vT")
                v_aug = work.tile([128, 16 * (HD + 1)], BF16, tag="vaug")
                nc.vector.memset(v_aug[:], 1.0)

                # kv proj (merged: wk|wv adjacent -> kT rows 0:64, vT rows
                # 64:128 of one psum tile) + k rope + V transposes
                def emit_kvproj(t4):
                    sl = slice(512 * t4, 512 * t4 + 512)
                    ps = psp.tile([128, 512], F32, tag="sc", bufs=SCB)
                    for c in range(8):
                        nc.tensor.matmul(ps[:], wkv_sb[c],
                                         xbf[c][:, sl],
                                         start=(c == 0), stop=(c == 7))
                    k_raw = work.tile([64, 512], BF16, tag="kraw", bufs=2)
                    nc.vector.tensor_copy(k_raw[:], ps[0:64, :])
                    nc.vector.tensor_copy(vT_sb[64:128, sl], ps[64:128, :])
                    sw = psp.tile([64, 512], F32, tag="sc", bufs=SCB)
                    nc.tensor.matmul(sw[:], perm_sb[0:64, 0:64],
                                     k_raw[:], start=True, stop=True)
                    t1 = work.tile([64, 512], BF16, tag="t1k", bufs=2)
                    nc.vector.tensor_mul(t1[:], k_raw[:], cos_sb[0:64, sl])
                    t2 = work.tile([64, 512], BF16, tag="t2k", bufs=2)
                    nc.vector.tensor_mul(t2[:], sw[:], sin_sb[0:64, sl])
                    nc.vector.tensor_add(k_rot[0:64, sl], t1[:], t2[:])
                    # duplicate k rows for the hb=64 head slots
                    nc.sync.dma_start(k_rot[64:128, sl], k_rot[0:64, sl])
                    # V transposes for this chunk (4 k-blocks)
                    for j in range(4 * t4, 4 * t4 + 4):
                        tp = psp.tile([128, 64], BF16, tag="sc", bufs=SCB)
                        nc.tensor.transpose(
                            tp[:],
                            vT_sb[64:128, 128 * j:128 * j + 128],
                            ident128[64:128, :])
                        nc.vector.tensor_copy(v_aug[:, 65 * j:65 * j + 64],
                                              tp[:])

                # q proj + rope, one 128-row chunk (= 2 heads) at a time
                def emit_qproj(rc, t4s):
                    for t4 in t4s:
                        sl = slice(512 * t4, 512 * t4 + 512)
                        ps = psp.tile([128, 512], F32, tag="sc", bufs=SCB)
                        for c in range(8):
                            nc.tensor.matmul(
                                ps[:],
                                wq_sb[c][:, 128 * rc:128 * rc + 128],
                                xbf[c][:, sl],
                                start=(c == 0), stop=(c == 7))
                        q_raw = work.tile([128, 512], BF16, tag="qraw",
                                          bufs=2)
                        nc.vector.tensor_copy(q_raw[:], ps[:])
                        sw = psp.tile([128, 512], F32, tag="sc", bufs=SCB)
                        nc.tensor.matmul(sw[:], perm_sb[:], q_raw[:],
                                         start=True, stop=True)
                        t1 = work.tile([128, 512], BF16, tag="t1", bufs=2)
                        nc.vector.tensor_mul(t1[:], q_raw[:],
                                             cos_sb[:, sl])
                        t2 = work.tile([128, 512], BF16, tag="t2", bufs=2)
                        nc.vector.tensor_mul(t2[:], sw[:], sin_sb[:, sl])
                        nc.vector.tensor_add(q_rot[rc][:, sl], t1[:], t2[:])

                # ---- attention: one head, one token half (1024 q cols)
                def emit_head(qh, h):
                    jmax = 8 * (qh + 1)
                    hb = 64 * (h % 2)
                    q_h = q_rot[h // 2]
                    attn_ps = psp.tile([65, 1024], F32, tag="attn", bufs=1)

                    def emit_pv(pv):
                        pt_, q0_, j_ = pv
                        for r in range(2):
                            rs = 1024 * qh + 512 * r
                            s0 = max(q0_, rs)
                            s1 = rs + 512
                            if s0 >= s1:
                                continue
                            nc.tensor.matmul(
                                attn_ps[:, s0 - 1024 * qh:
                                        s1 - 1024 * qh],
                                v_aug[:, 65 * j_:65 * j_ + 65],
                                pt_[:, s0 - q0_:s1 - q0_],
                                start=(j_ == 0),
                                stop=(j_ == 8 * qh + 4 * r + 3))

                    pend = []
                    for j in range(jmax):
                        q0 = max(1024 * qh, 128 * j)
                        q1 = 1024 * (qh + 1)
                        qlen = q1 - q0
                        sc = psp.tile([128, 1024], F32, tag="sc", bufs=SCB)
                        off = 0
                        while off < qlen:
                            n = min(512, qlen - off)
                            nc.tensor.matmul(
                                sc[:, off:off + n],
                                k_rot[hb:hb + 64,
                                      128 * j:128 * j + 128],
                                q_h[hb:hb + 64,
                                    q0 + off:q0 + off + n],
                                start=True, stop=True)
                            off += n
                        pt = work.tile([128, 1024], BF16, tag="pt",
                                       bufs=4)
                        nc.scalar.activation(pt[:, 0:qlen],
                                             sc[:, 0:qlen],
                                             FP.Exp, scale=0.125)
                        if 128 * j >= 1024 * qh:
                            nc.vector.tensor_mul(pt[:, 0:128],
                                                 pt[:, 0:128],
                                                 tri_sb[:])
                        pend.append((pt, q0, j))
                        if len(pend) >= 3:
                            emit_pv(pend.pop(0))
                    for pv in pend:
                        emit_pv(pv)
                    # evacuate PSUM early (frees the attn slot), then
                    # normalize: anorm = attn[0:64] * bcast(1/Z)
                    acopy = work.tile([65, 1024], BF16, tag="acopy",
                                      bufs=2)
                    nc.vector.tensor_copy(acopy[:], attn_ps[:])
                    zr1 = work.tile([1, 1024], BF16, tag="zr1", bufs=2)
                    with nc.allow_low_precision(
                            reason="bf16 1/Z; rel-err budget 2e-2"):
                        nc.vector.reciprocal(zr1[:], acopy[64:65, :])
                    zr = work.tile([64, 1024], BF16, tag="zr", bufs=2)
                    nc.gpsimd.partition_broadcast(zr[:], zr1[:])
                    with nc.allow_low_precision(
                            reason="bf16 softmax normalize"):
                        nc.vector.tensor_mul(acopy[0:64, :],
                                             acopy[0:64, :], zr[:])
                    nc.sync.dma_start(
                        cc_in[qh][64 * h:64 * h + 64, :],
                        acopy[0:64, :])

                def emit_ag(qh):
                    if no_collective:
                        for g4 in range(TPG):
                            nc.sync.dma_start(
                                cc_out[qh][QR * g4:QR * (g4 + 1), :],
                                cc_in[qh][:, :])
                    else:
                        nc.gpsimd.collective_compute(
                            "AllGather", mybir.AluOpType.bypass,
                            replica_groups=groups,
                            ins=[cc_in[qh].opt()],
                            outs=[cc_out[qh].opt()])

                oproj_loaded = {}

                def emit_oproj_load(qh):
                    attn_all = []
                    for c2 in range(8):
                        t = work.tile([128, 1024], BF16, tag=f"aall_{c2}",
                                      bufs=2)
                        nc.sync.dma_start(
                            t[:], cc_out[qh][128 * c2:128 * c2 + 128, :])
                        attn_all.append(t)
                    oproj_loaded[qh] = attn_all

                def emit_oproj(qh, tc8s):
                    attn_all = oproj_loaded[qh]
                    for tc8 in tc8s:
                        ps = psp.tile([128, OSL], F32, tag="sc", bufs=SCB)
                        for c2 in range(8):
                            nc.tensor.matmul(
                                ps[:],
                                attn_all[c2][:, 128 * tc8:128 * tc8 + 128],
                                wo_sb[c2],
                                start=(c2 == 0), stop=(c2 == 7))
                        ot = work.tile([128, OSL], F32, tag="ot", bufs=2)
                        nc.vector.tensor_copy(ot[:], ps[:])
                        nc.sync.dma_start(
                            out.ap()[1024 * qh + 128 * tc8:
                                     1024 * qh + 128 * tc8 + 128, :],
                            ot[:])

                # schedule: kv for first token half, q rc0 first half,
                # then qh0 attention can start while the rest projects.
                emit_kvproj(0)
                emit_kvproj(1)
                emit_qproj(0, [0, 1])
                emit_qproj(1, [0, 1])
                if tail_pending[0]:
                    emit_tail1()      # previous rep's qh=1 oproj
                emit_head(0, 0)
                emit_kvproj(2)
                emit_kvproj(3)
                emit_head(0, 1)
                emit_qproj(0, [2, 3])
                emit_head(0, 2)
                emit_qproj(1, [2, 3])
                emit_head(0, 3)
                emit_ag(0)
                emit_oproj_load(0)   # DMAs wait on AG0 asynchronously
                emit_head(1, 0)
                emit_head(1, 1)
                emit_oproj(0, range(4))
                emit_head(1, 2)
                emit_oproj(0, range(4, 8))
                emit_head(1, 3)
                emit_ag(1)
                tail_pending[0] = True

            emit_tail1()

    nc.compile()
    return nc


def _prep_inputs(x, cos, sin, wq, wk, wv, wo):
    x = np.ascontiguousarray(x, np.float32)
    cos = np.asarray(cos, np.float32)
    sin = np.asarray(sin, np.float32)
    wq = np.asarray(wq, np.float32)
    wk = np.asarray(wk, np.float32)
    wv = np.asarray(wv, np.float32)
    wo = np.asarray(wo, np.float32)

    sinp = np.concatenate([-sin[:, :HD // 2], sin[:, HD // 2:]], axis=1)
    cosT_np = np.ascontiguousarray(np.tile(cos.T, (2, 1)))        # [128, S]
    sinT_np = np.ascontiguousarray(np.tile(sinp.T, (2, 1)))       # [128, S]
    perm = np.zeros((128, 128), np.float32)
    for i in range(128):
        perm[i, (i + 32) % 64 + 64 * (i // 64)] = 1.0
    permT_np = np.ascontiguousarray(perm.T)
    tri_np = (np.arange(128)[:, None] <= np.arange(128)[None, :]) \
        .astype(np.float32)

    BFN = ml_dtypes.bfloat16
    in_maps = []
    for c in range(NCORES):
        b, g = c // TPG, c % TPG
        in_maps.append({
            "xT": np.ascontiguousarray(x[b].T).astype(BFN),
            "permT": permT_np.astype(BFN),
            "wq": np.ascontiguousarray(wq[:, QR * g:QR * (g + 1)]).astype(BFN),
            "wk": np.ascontiguousarray(wk[:, HD * g:HD * (g + 1)]).astype(BFN),
            "wv": np.ascontiguousarray(wv[:, HD * g:HD * (g + 1)]).astype(BFN),
            "wo": np.ascontiguousarray(wo[:, OSL * g:OSL * (g + 1)]).astype(BFN),
            "cosT": cosT_np.astype(BFN),
            "sinT": sinT_np.astype(BFN),
            "tri": tri_np.astype(BFN),
            "ident": np.eye(64, dtype=BFN),
        })
    return in_maps


def kernel(x, cos, sin, wq, wk, wv, wo):
    global LAST_RESULTS
    if "nc" not in _CACHE:
        _CACHE["nc"] = _build()
    nc = _CACHE["nc"]
    in_maps = _prep_inputs(x, cos, sin, wq, wk, wv, wo)
    res = bass_utils.run_bass_kernel_spmd(
        nc, in_maps, core_ids=list(range(NCORES)), trace=PROFILE)
    LAST_RESULTS = res
    outs = [res.results[c]["out"] for c in range(NCORES)]
    full = np.stack([
        np.concatenate([outs[TPG * b + g] for g in range(TPG)], axis=1)
        for b in range(B)
    ]).astype(np.float32)
    return full
